# revision 1
# baseline (speedup 1.0000x reference)
"""Causal self-attention (S=8192, d_model=1024, d_k=d_v=128) on 8 TRN2 cores.

Sharding: q-row tiles interleaved over cores (core m owns global 128-row
tiles t = m+8j, j=0..7) -> identical static SPMD program per core with
balanced causal work; per-core mask tables handle the diagonal.
K^T/V computed per-core for its own rows, AllGathered to all cores.
Scores computed transposed (S_T[k,q]) so exp runs ACT PSUM->SBUF, PV
consumes probs_T directly (V-block stationary), row sums via ones-matmul,
Z^T accumulated in PSUM, transposed at the end and scaled by 1/rowsum.
"""
import contextlib

import numpy as np

import concourse.bass as bass
import concourse.mybir as mybir
import concourse.tile as tile
from concourse import bacc
from concourse.bass_utils import run_bass_kernel_spmd
from concourse.masks import make_identity

F32 = mybir.dt.float32
F32R = mybir.dt.float32r

S = 8192
D = 1024
DK = 128
NCORES = 8
NT = S // 128          # 64 global k/q tiles
JT = NT // NCORES      # 8 local q-tiles per core
SCALE = 1.0 / np.sqrt(128.0)

DT_MM = F32R           # matmul compute dtype knob (F32R fast / F32 exact)

_BUILT = {}


def _abs_chunks(qlo):
    """Pieces of [qlo, 1024) that never cross a 512 (PSUM bank) boundary."""
    if qlo < 512:
        return [(qlo, 512 - qlo), (512, 512)]
    return [(qlo, 1024 - qlo)]


def build(rep: int = 1):
    nc = bacc.Bacc("TRN2", target_bir_lowering=False, debug=False)

    XTQ = nc.declare_dram_parameter("XTQ", [8, 128, 1024], DT_MM, isOutput=False)
    WQT = nc.declare_dram_parameter("WQT", [8, 128, 128], DT_MM, isOutput=False)
    WKT = nc.declare_dram_parameter("WKT", [8, 128, 128], DT_MM, isOutput=False)
    WVT = nc.declare_dram_parameter("WVT", [8, 128, 128], DT_MM, isOutput=False)
    MASKS = nc.declare_dram_parameter("MASKS", [8, 128, 128], DT_MM, isOutput=False)
    ZOUT = nc.declare_dram_parameter("ZOUT", [JT, 128, 128], F32, isOutput=True)

    with tile.TileContext(nc) as tc:
        ctx = contextlib.ExitStack()
        with ctx:
            sb = ctx.enter_context(tc.tile_pool(name="sb", bufs=1))
            # ---- persistent inputs ----
            xq = sb.tile([128, 8, 1024], DT_MM)
            for _h in range(2):
                nc.sync.dma_start(
                    out=xq[:, :, _h * 512:(_h + 1) * 512],
                    in_=XTQ[:].rearrange("c p n -> p c n")[:, :, _h * 512:(_h + 1) * 512])
            wq = sb.tile([128, 8, 128], DT_MM)
            wk = sb.tile([128, 8, 128], DT_MM)
            wv = sb.tile([128, 8, 128], DT_MM)
            nc.sync.dma_start(out=wq, in_=WQT[:].rearrange("c p n -> p c n"))
            nc.sync.dma_start(out=wk, in_=WKT[:].rearrange("c p n -> p c n"))
            nc.sync.dma_start(out=wv, in_=WVT[:].rearrange("c p n -> p c n"))
            masks = sb.tile([128, 8, 128], DT_MM)
            nc.sync.dma_start(out=masks, in_=MASKS[:].rearrange("r p n -> p r n"))

            ident_f = sb.tile([128, 128], F32)
            make_identity(nc, ident_f)
            ident = sb.tile([128, 128], DT_MM)
            nc.vector.tensor_copy(ident, ident_f)
            ones_f = sb.tile([128, 1], F32)
            nc.vector.memset(ones_f, 1.0)
            ones = sb.tile([128, 1], DT_MM)
            nc.vector.tensor_copy(ones, ones_f)

            qt = sb.tile([128, 1024], DT_MM)       # Q^T local
            ktl = sb.tile([128, 1024], DT_MM)      # K^T local
            vt = sb.tile([128, 1024], DT_MM)       # V^T local
            vl = sb.tile([128, 8, 128], DT_MM)     # V local, normal orient
            # gathered K^T / V, split in k-halves for collective overlap
            ktsA = sb.tile([128, NT // 2, 128], DT_MM)
            ktsB = sb.tile([128, NT // 2, 128], DT_MM)
            vsA = sb.tile([128, NT // 2, 128], DT_MM)
            vsB = sb.tile([128, NT // 2, 128], DT_MM)
            zt_sb = sb.tile([128, 1024], DT_MM)
            sums_sb = sb.tile([1, 1024], F32)
            sums_t = sb.tile([128, 8], F32)
            recip = sb.tile([128, 8], F32)
            zo = sb.tile([128, 8, 128], F32)

            for _r in range(rep):
                # DRAM bounce + gather buffers (per rep for timing builds)
                kt_bounceA = nc.dram_tensor(f"kt_bounceA{_r}", [128, 512], DT_MM)
                kt_bounceB = nc.dram_tensor(f"kt_bounceB{_r}", [128, 512], DT_MM)
                v_bounce = nc.dram_tensor(f"v_bounce{_r}", [1024, 128], DT_MM)
                kt_gathA = nc.dram_tensor(f"kt_gathA{_r}", [8, 128, 512], DT_MM,
                                          addr_space="Shared")
                kt_gathB = nc.dram_tensor(f"kt_gathB{_r}", [8, 128, 512], DT_MM,
                                          addr_space="Shared")
                v_gathA = nc.dram_tensor(f"v_gathA{_r}", [8, 512, 128], DT_MM,
                                         addr_space="Shared")
                v_gathB = nc.dram_tensor(f"v_gathB{_r}", [8, 512, 128], DT_MM,
                                         addr_space="Shared")
                rg = [list(range(NCORES))]

                # ---- projections: K first, V second (feed collectives
                # ASAP), Q last (overlaps with the gathers) ----
                with tc.tile_pool(name=f"pp{_r}", bufs=2, space="PSUM") as pp:
                    for h in range(2):
                        pk = pp.tile([128, 512], F32, tag="proj")
                        for c in range(8):
                            nc.tensor.matmul(
                                pk, lhsT=wk[:, c],
                                rhs=xq[:, c, h * 512:(h + 1) * 512],
                                start=(c == 0), stop=(c == 7))
                        nc.scalar.copy(ktl[:, h * 512:(h + 1) * 512], pk)
                        nc.sync.dma_start(
                            out=(kt_bounceA if h == 0 else kt_bounceB)[:],
                            in_=ktl[:, h * 512:(h + 1) * 512])
                    nc.gpsimd.collective_compute(
                        "AllGather", mybir.AluOpType.bypass, replica_groups=rg,
                        ins=[kt_bounceA[:]], outs=[kt_gathA[:]])

                    for h in range(2):
                        pv = pp.tile([128, 512], F32, tag="proj")
                        for c in range(8):
                            nc.tensor.matmul(
                                pv, lhsT=wv[:, c],
                                rhs=xq[:, c, h * 512:(h + 1) * 512],
                                start=(c == 0), stop=(c == 7))
                        nc.scalar.copy(vt[:, h * 512:(h + 1) * 512], pv)
                        for j in range(4 * h, 4 * h + 4):
                            pt = pp.tile([128, 128], F32, tag="tr")
                            nc.tensor.matmul(
                                pt, lhsT=vt[:, j * 128:(j + 1) * 128],
                                rhs=ident, start=True, stop=True)
                            nc.vector.tensor_copy(vl[:, j], pt)
                        nc.sync.dma_start(
                            out=v_bounce[:].rearrange(
                                "(j p) v -> p j v", p=128)[:, 4 * h:4 * h + 4],
                            in_=vl[:, 4 * h:4 * h + 4])
                    nc.gpsimd.collective_compute(
                        "AllGather", mybir.AluOpType.bypass, replica_groups=rg,
                        ins=[v_bounce[0:512]], outs=[v_gathA[:]])
                    nc.gpsimd.collective_compute(
                        "AllGather", mybir.AluOpType.bypass, replica_groups=rg,
                        ins=[kt_bounceB[:]], outs=[kt_gathB[:]])
                    nc.gpsimd.collective_compute(
                        "AllGather", mybir.AluOpType.bypass, replica_groups=rg,
                        ins=[v_bounce[512:1024]], outs=[v_gathB[:]])

                    # Q^T projection overlaps the gathers
                    for h in range(2):
                        pq = pp.tile([128, 512], F32, tag="proj")
                        for c in range(8):
                            nc.tensor.matmul(
                                pq, lhsT=wq[:, c],
                                rhs=xq[:, c, h * 512:(h + 1) * 512],
                                start=(c == 0), stop=(c == 7))
                        nc.scalar.copy(qt[:, h * 512:(h + 1) * 512], pq)

                # gathered -> SBUF (k-tile t at source core t%8, slice t//8)
                for c in range(8):
                    nc.sync.dma_start(
                        out=ktsA[:, 4 * c:4 * c + 4, :],
                        in_=kt_gathA[c].rearrange("p (j n) -> p j n", n=128))
                    nc.sync.dma_start(
                        out=vsA[:, 4 * c:4 * c + 4, :],
                        in_=v_gathA[c].rearrange("(j p) v -> p j v", p=128))
                for c in range(8):
                    nc.sync.dma_start(
                        out=ktsB[:, 4 * c:4 * c + 4, :],
                        in_=kt_gathB[c].rearrange("p (j n) -> p j n", n=128))
                    nc.sync.dma_start(
                        out=vsB[:, 4 * c:4 * c + 4, :],
                        in_=v_gathB[c].rearrange("(j p) v -> p j v", p=128))

                # ---- attention: k-outer loop ----
                with tc.tile_pool(name=f"psc{_r}", bufs=4, space="PSUM") as psc, \
                     tc.tile_pool(name=f"pzt{_r}", bufs=1, space="PSUM") as pzt, \
                     tc.tile_pool(name=f"psm{_r}", bufs=1, space="PSUM") as psm, \
                     tc.tile_pool(name=f"prb{_r}", bufs=6) as prb:
                    zt_ps = pzt.tile([128, 1024], F32)
                    sums_ps = psm.tile([1, 1024], F32)
                    for kt in range(NT):
                        g = kt // 8
                        r = kt % 8
                        qlo = 128 * g
                        kl = kt if kt < NT // 2 else kt - NT // 2
                        idx = 4 * (kl % 8) + kl // 8
                        if kt < NT // 2:
                            ktile, vtile = ktsA[:, idx, :], vsA[:, idx, :]
                        else:
                            ktile, vtile = ktsB[:, idx, :], vsB[:, idx, :]
                        for off, n in _abs_chunks(qlo):
                            sc = psc.tile([128, 512], F32, tag="sc")
                            nc.tensor.matmul(
                                sc[:, 0:n], lhsT=ktile,
                                rhs=qt[:, off:off + n],
                                start=True, stop=True)
                            pr = prb.tile([128, 512], DT_MM, tag="pr")
                            nc.scalar.activation(
                                out=pr[:, 0:n], in_=sc[:, 0:n],
                                func=mybir.ActivationFunctionType.Exp,
                                scale=SCALE)
                            if off == qlo:
                                nc.vector.tensor_mul(pr[:, 0:128],
                                                     pr[:, 0:128],
                                                     masks[:, r, :])
                            nc.tensor.matmul(
                                zt_ps[:, off:off + n],
                                lhsT=vtile, rhs=pr[:, 0:n],
                                start=(kt == 0), stop=(kt == NT - 1),
                                skip_group_check=True)
                            nc.tensor.matmul(
                                sums_ps[:, off:off + n],
                                lhsT=ones, rhs=pr[:, 0:n],
                                start=(kt == 0), stop=(kt == NT - 1),
                                skip_group_check=True)

                    nc.scalar.copy(zt_sb, zt_ps)
                    nc.vector.tensor_copy(sums_sb, sums_ps)

                # ---- finalize ----
                sums_scratch = nc.dram_tensor(f"sums_scratch{_r}", [1024], F32)
                nc.sync.dma_start(
                    out=sums_scratch[:].rearrange("(o n) -> o n", o=1),
                    in_=sums_sb)
                nc.sync.dma_start(
                    out=sums_t,
                    in_=sums_scratch[:].rearrange("(j p) -> p j", p=128))
                nc.vector.reciprocal(recip, sums_t)
                with tc.tile_pool(name=f"ptr{_r}", bufs=2, space="PSUM") as ptr:
                    for j in range(JT):
                        pt = ptr.tile([128, 128], F32, tag="ztr")
                        nc.tensor.matmul(
                            pt, lhsT=zt_sb[:, j * 128:(j + 1) * 128],
                            rhs=ident, start=True, stop=True)
                        nc.vector.tensor_scalar_mul(zo[:, j], pt,
                                                    recip[:, j:j + 1])
                nc.sync.dma_start(out=ZOUT[:].rearrange("j p v -> p j v"),
                                  in_=zo)

    nc.compile()
    return nc


def _host_prep(X, Wq, Wk, Wv):
    X = np.asarray(X, np.float32)
    XT = np.ascontiguousarray(X.T)                           # [1024, 8192]
    wqt = np.ascontiguousarray(np.asarray(Wq, np.float32).T).reshape(8, 128, 128)
    wkt = np.ascontiguousarray(np.asarray(Wk, np.float32).T).reshape(8, 128, 128)
    wvt = np.ascontiguousarray(np.asarray(Wv, np.float32).T).reshape(8, 128, 128)
    tri = np.triu(np.ones((128, 128), np.float32))           # 1 if k<=q
    in_maps = []
    for m in range(NCORES):
        cols = np.concatenate(
            [np.arange((m + 8 * j) * 128, (m + 8 * j + 1) * 128)
             for j in range(JT)])
        xtq = np.ascontiguousarray(XT[:, cols]).reshape(8, 128, 1024)
        masks = np.zeros((8, 128, 128), np.float32)
        for r in range(8):
            if r < m:
                masks[r] = 1.0
            elif r == m:
                masks[r] = tri
        in_maps.append({"XTQ": xtq, "WQT": wqt, "WKT": wkt, "WVT": wvt,
                        "MASKS": masks})
    return in_maps


def kernel(X, Wq, Wk, Wv):
    if "nc" not in _BUILT:
        _BUILT["nc"] = build()
    nc = _BUILT["nc"]
    in_maps = _host_prep(X, Wq, Wk, Wv)
    res = run_bass_kernel_spmd(nc, in_maps, list(range(NCORES)))
    Z = np.empty((S, 128), np.float32)
    for m in range(NCORES):
        zo = res.results[m]["ZOUT"]                # [JT, 128, 128]
        for j in range(JT):
            t = m + 8 * j
            Z[t * 128:(t + 1) * 128, :] = zo[j]
    return Z

